# revision 4
# baseline (speedup 1.0000x reference)
"""Trainium2 Bass kernel for nn_InterAttention — v2 (G-trick).

Algebra: only pooled outputs are needed. With A = relu(r1@W1.T+b1),
C = relu(r2@W1.T+b1), G = W2.T@W2, u = b2@W2:
    scores S[i,j] = (A@G)[i]·C[j] + (A·u)[i] + (C·u)[j] + const
    r1_pool = [abar@W2.T+b2, q1@W2.T+b2],  abar = mean_i A, q1 = (w1@C)/L1
    r2_pool = [cbar@W2.T+b2, q2@W2.T+b2],  cbar = mean_j C, q2 = (w2@A)/L2
so the per-row W2 layer is replaced by one A@G per batch plus a single
32-column W2 projection at the end. Plain pools come free as accum_out
on the layer-1 PSUM evacuation.
"""

import numpy as np

import concourse.bacc as bacc
import concourse.mybir as mybir
import concourse.tile as tile
from concourse.bass_utils import run_bass_kernel_spmd

L1, L2, B, D = 256, 320, 64, 1024
NCORES = 8
BL = B // NCORES            # batches per core
NB = L1 + L2                # rows per batch (r1 cols then r2 cols)
KT = D // 128               # contraction tiles
F32 = mybir.dt.float32
F32R = mybir.dt.float32r
BF16 = mybir.dt.bfloat16

MM_DTYPE = "bf16"


def build_kernel(mm_dtype=MM_DTYPE):
    mm_dt = {"f32r": F32R, "f32": F32, "bf16": BF16}[mm_dtype]
    act_dt = mm_dt

    def f32v(ap):
        # non-matmul engines read f32r bytes as plain f32
        return ap.bitcast(F32) if mm_dtype == "f32r" else ap

    nc = bacc.Bacc("TRN2", target_bir_lowering=False, debug=False)

    xT = nc.dram_tensor("xT", [128, KT, BL * NB], mm_dt, kind="ExternalInput")
    w1T = nc.dram_tensor("w1T", [128, KT, D], mm_dt, kind="ExternalInput")
    gT = nc.dram_tensor("gT", [128, KT, D], mm_dt, kind="ExternalInput")
    w2T = nc.dram_tensor("w2T", [128, KT, D], mm_dt, kind="ExternalInput")
    b1d = nc.dram_tensor("b1d", [128, KT], F32, kind="ExternalInput")
    b2d = nc.dram_tensor("b2d", [128, KT], F32, kind="ExternalInput")
    uDd = nc.dram_tensor("uDd", [128, KT], mm_dt, kind="ExternalInput")
    IDd = nc.dram_tensor("IDd", [128, 128], mm_dt, kind="ExternalInput")
    # outputs in SBUF-native (p, b, f) layout; host un-permutes
    out1 = nc.dram_tensor("out1", [128, BL, 2 * KT], F32, kind="ExternalOutput")
    out2 = nc.dram_tensor("out2", [128, BL, 2 * KT], F32, kind="ExternalOutput")

    CH = [(0, L1), (L1, L2)]        # layer-1 chunks align with A|C split

    with tile.TileContext(nc) as tc:
        with (
            tc.tile_pool(name="wpool", bufs=1) as wpool,
            tc.tile_pool(name="xpool", bufs=2) as xpool,
            tc.tile_pool(name="hpool", bufs=2) as hpool,
            tc.tile_pool(name="gpool", bufs=2) as gpool,
            tc.tile_pool(name="spool", bufs=6) as spool,
            tc.tile_pool(name="stat", bufs=16) as stat,
            tc.tile_pool(name="opool", bufs=1) as opool,
            tc.tile_pool(name="mmps", bufs=2, space="PSUM") as mmps,
            tc.tile_pool(name="atps", bufs=2, space="PSUM") as atps,
            tc.tile_pool(name="wrps", bufs=2, space="PSUM") as wrps,
            tc.tile_pool(name="wbps", bufs=2, space="PSUM") as wbps,
        ):
            # --- resident weights ---
            w1s = wpool.tile([128, KT, D], act_dt, name="w1s")
            gs = wpool.tile([128, KT, D], act_dt, name="gs")
            w2s = wpool.tile([128, KT, D], act_dt, name="w2s")
            b1s = wpool.tile([128, KT], F32, name="b1s")
            b2s = wpool.tile([128, KT], F32, name="b2s")
            us = wpool.tile([128, KT], act_dt, name="us")
            ids = wpool.tile([128, 128], act_dt, name="ids")
            ones1 = wpool.tile([1, L2], act_dt, name="ones1")
            ones1f = wpool.tile([1, L2], F32, name="ones1f")
            onesC = wpool.tile([128, 1], act_dt, name="onesC")
            onesCf = wpool.tile([128, 1], F32, name="onesCf")
            nc.sync.dma_start(out=b1s[:], in_=b1d[:])
            nc.sync.dma_start(out=b2s[:], in_=b2d[:])
            nc.sync.dma_start(out=us[:], in_=uDd[:])
            nc.sync.dma_start(out=ids[:], in_=IDd[:])
            # w1 split per m-slice so the first MM group starts early;
            # gs/w2s are deferred below (first needed one layer / 8 batches in)
            for m in range(KT):
                nc.sync.dma_start(out=w1s[:, :, m * 128:(m + 1) * 128],
                                  in_=w1T[:, :, m * 128:(m + 1) * 128])
            nc.vector.memset(ones1f[:], 1.0)
            nc.vector.tensor_copy(ones1[:], ones1f[:])
            nc.vector.memset(onesCf[:], 1.0)
            nc.vector.tensor_copy(onesC[:], onesCf[:])

            # pooled D-vectors, written directly in projection layout
            # [128, KT, BL, v] with v in (abar_raw, q1_raw, cbar_raw, q2_raw);
            # the 1/L1 (resp 1/L2) scales are folded into the final ACT evac.
            Vr = opool.tile([128, KT, BL, 4], act_dt, name="Vr")

            NIT = L1 // 128

            # ---- emission helpers: phase A (PE-dense) / phase B (dep chains).
            # Phase B of batch b is emitted interleaved into batch b+1's
            # layer-1 quarters so the PE never waits on DVE/ACT round-trips.

            def layer1_part(b, H, xcs, part):
                ci, half = divmod(part, 2)
                c0, cw = CH[ci]
                cb = b * NB
                if half == 0:
                    xc = xpool.tile([128, KT, cw], act_dt, name="xc", tag="xc")
                    # ACT-engine HWDGE queue, parallel to SP weight queue
                    nc.scalar.dma_start(
                        out=xc[:], in_=xT[:, :, cb + c0: cb + c0 + cw])
                    xcs[ci] = xc
                xc = xcs[ci]
                v = 0 if ci == 0 else 2
                for m in range(half * 4, half * 4 + 4):
                    ps = mmps.tile([128, cw], F32, name="ps1", tag="mm")
                    for k in range(KT):
                        nc.tensor.matmul(
                            ps[:], w1s[:, k, m * 128:(m + 1) * 128],
                            xc[:, k, :], start=(k == 0), stop=(k == KT - 1))
                    with nc.allow_low_precision(reason="pooled sums in bf16"):
                        nc.scalar.activation(
                            H[:, m, c0:c0 + cw], ps[:],
                            mybir.ActivationFunctionType.Relu,
                            bias=b1s[:, m:m + 1], scale=1.0,
                            accum_out=Vr[:, m, b, v:v + 1])

            def phaseA_rest(b, H):
                if b == 0:
                    for m in range(KT):
                        nc.sync.dma_start(out=gs[:, :, m * 128:(m + 1) * 128],
                                          in_=gT[:, :, m * 128:(m + 1) * 128])
                if b == 1:
                    nc.sync.dma_start(out=w2s[:], in_=w2T[:])

                # layer G: AG = G @ A_T (+u per-partition via bias)
                AG = gpool.tile([128, KT, L1], act_dt, name="AG", tag="AG")
                for m in range(KT):
                    ps = mmps.tile([128, L1], F32, name="psg", tag="mm")
                    for k in range(KT):
                        nc.tensor.matmul(
                            ps[:], gs[:, k, m * 128:(m + 1) * 128],
                            H[:, k, 0:L1], start=(k == 0), stop=(k == KT - 1))
                    nc.scalar.activation(
                        AG[:, m, :], ps[:], mybir.ActivationFunctionType.Identity,
                        bias=f32v(us[:, m:m + 1]), scale=1.0)

                # Au row: Au[i] = sum_d u[d] A_T[d, i]
                psAu = wrps.tile([1, L1], F32, name="psAu", tag="wrow")
                for k in range(KT):
                    nc.tensor.matmul(
                        psAu[:], us[:, k:k + 1], H[:, k, 0:L1],
                        start=(k == 0), stop=(k == KT - 1))
                AuRow = stat.tile([1, L1], act_dt, name="AuRow", tag="wrow_sb",
                                  bufs=3)
                nc.vector.tensor_copy(AuRow[:], psAu[:])

                # scores T = AG·C^T + Cu[j] + Au[i], tiles [i, j].
                # |T| << 80 so exp() skips max-subtraction; E1 = exp(T) serves
                # BOTH softmaxes: w1[j]=sum_i E1/Z1[i], w2[i]=sum_j E1/Z2[j]
                evs, rss = [], []
                for it in range(NIT):
                    po = atps.tile([128, L2], F32, name="po", tag="po")
                    for k in range(KT):
                        nc.tensor.matmul(
                            po[:], AG[:, k, it * 128:(it + 1) * 128],
                            H[:, k, L1:NB], start=(k == 0), stop=False)
                    nc.tensor.matmul(po[:], AuRow[:, it * 128:(it + 1) * 128],
                                     ones1[:], start=False, stop=True)
                    ev = spool.tile([128, L2], act_dt, name="ev", tag="e1",
                                    bufs=4)
                    ssum = stat.tile([128, 1], F32, name="ssum", tag="st")
                    nc.scalar.activation(ev[:], po[:],
                                         mybir.ActivationFunctionType.Exp,
                                         scale=1.0, accum_out=ssum[:])
                    rs = stat.tile([128, 1], act_dt, name="rs", tag="st")
                    with nc.allow_low_precision(reason="softmax 1/sum"):
                        nc.vector.reciprocal(rs[:], ssum[:])
                    evs.append(ev)
                    rss.append(rs)
                return {"b": b, "H": H, "evs": evs, "rss": rss}

            def emit_B1(st):
                # weight-sum rows: w1ps = sum_i E1/Z1; Z2ps = colsum E1
                w1ps = wrps.tile([1, L2], F32, name="w1ps", tag="wrow")
                for it in range(NIT):
                    nc.tensor.matmul(w1ps[:], st["rss"][it][:], st["evs"][it][:],
                                     start=(it == 0), stop=(it == NIT - 1))
                Z2ps = wrps.tile([1, L2], F32, name="Z2ps", tag="wrow")
                for it in range(NIT):
                    nc.tensor.matmul(Z2ps[:], onesC[:], st["evs"][it][:],
                                     start=(it == 0), stop=(it == NIT - 1))
                w1row = stat.tile([1, L2], act_dt, name="w1row", tag="wrow_sb",
                                  bufs=3)
                nc.vector.tensor_copy(w1row[:], w1ps[:])
                Z2inv = stat.tile([1, L2], act_dt, name="Z2inv", tag="wrow_sb",
                                  bufs=3)
                with nc.allow_low_precision(reason="softmax 1/sum"):
                    nc.vector.reciprocal(Z2inv[:], Z2ps[:])
                st["w1row"], st["Z2inv"] = w1row, Z2inv

            def emit_B2(st):
                w1b = wbps.tile([128, L2], F32, name="w1b", tag="wb")
                nc.tensor.matmul(w1b[:], ones1[:, 0:128], st["w1row"][:],
                                 start=True, stop=True)
                Z2b = wbps.tile([128, L2], F32, name="Z2b", tag="wb")
                nc.tensor.matmul(Z2b[:], ones1[:, 0:128], st["Z2inv"][:],
                                 start=True, stop=True)
                w2cBs = []
                for it in range(NIT):
                    junk = spool.tile([128, L2], F32, name="junk2", tag="scratch")
                    w2cF = stat.tile([128, 1], F32, name="w2cF", tag="st")
                    nc.vector.scalar_tensor_tensor(
                        out=junk[:], in0=st["evs"][it][:], scalar=1.0, in1=Z2b[:],
                        op0=mybir.AluOpType.mult, op1=mybir.AluOpType.mult,
                        accum_out=w2cF[:])
                    w2cB = stat.tile([128, 1], act_dt, name="w2cB", tag="st")
                    nc.vector.tensor_copy(w2cB[:], w2cF[:])
                    w2cBs.append(w2cB)
                st["w1b"], st["w2cBs"] = w1b, w2cBs

            def emit_B3(st):
                b, H = st["b"], st["H"]
                # w2 columns -> row via PE transpose against identity
                w2row = stat.tile([1, L1], act_dt, name="w2row", tag="wrow_sb",
                                  bufs=3)
                for it in range(NIT):
                    w2rp = wrps.tile([1, 128], act_dt, name="w2rp", tag="wrow")
                    nc.tensor.transpose(w2rp[:], st["w2cBs"][it][:], ids[:])
                    nc.vector.tensor_copy(w2row[:, it * 128:(it + 1) * 128],
                                          w2rp[:])
                st["w2row"] = w2row
                for k in range(KT):
                    junk = spool.tile([128, L2], F32, name="junk1", tag="scratch")
                    with nc.allow_low_precision(reason="pooled sums in bf16"):
                        nc.vector.scalar_tensor_tensor(
                            out=junk[:], in0=f32v(H[:, k, L1:NB]),
                            scalar=1.0, in1=st["w1b"][:],
                            op0=mybir.AluOpType.mult, op1=mybir.AluOpType.mult,
                            accum_out=Vr[:, k, b, 1:2])

            def emit_B4(st):
                b, H = st["b"], st["H"]
                w2b = wbps.tile([128, L1], F32, name="w2b", tag="wb")
                nc.tensor.matmul(w2b[:], ones1[:, 0:128], st["w2row"][:],
                                 start=True, stop=True)
                for k in range(KT):
                    junk = spool.tile([128, L1], F32, name="junk3", tag="scratch")
                    with nc.allow_low_precision(reason="pooled sums in bf16"):
                        nc.vector.scalar_tensor_tensor(
                            out=junk[:], in0=f32v(H[:, k, 0:L1]),
                            scalar=1.0, in1=w2b[:],
                            op0=mybir.AluOpType.mult, op1=mybir.AluOpType.mult,
                            accum_out=Vr[:, k, b, 3:4])

            prev = None
            for b in range(BL):
                H = hpool.tile([128, KT, NB], act_dt, name="H", tag="H")
                xcs = {}
                layer1_part(b, H, xcs, 0)
                if prev: emit_B1(prev)
                layer1_part(b, H, xcs, 1)
                if prev: emit_B2(prev)
                layer1_part(b, H, xcs, 2)
                if prev: emit_B3(prev)
                layer1_part(b, H, xcs, 3)
                if prev: emit_B4(prev)
                prev = phaseA_rest(b, H)
            emit_B1(prev)
            emit_B2(prev)
            emit_B3(prev)
            emit_B4(prev)

            # --- final projection: out = W2 @ V + b2, /L folded into evac ---
            outF1 = opool.tile([128, BL, 2, KT], F32, name="outF1")
            outF2 = opool.tile([128, BL, 2, KT], F32, name="outF2")
            for m in range(KT):
                ps = mmps.tile([128, BL, 4], F32, name="psF", tag="mm")
                for k in range(KT):
                    nc.tensor.matmul(
                        ps[:, :, :], w2s[:, k, m * 128:(m + 1) * 128],
                        Vr[:, k, :, :], start=(k == 0), stop=(k == KT - 1))
                nc.scalar.activation(
                    outF1[:, :, :, m], ps[:, :, 0:2],
                    mybir.ActivationFunctionType.Identity,
                    bias=b2s[:, m:m + 1], scale=1.0 / L1)
                nc.scalar.activation(
                    outF2[:, :, :, m], ps[:, :, 2:4],
                    mybir.ActivationFunctionType.Identity,
                    bias=b2s[:, m:m + 1], scale=1.0 / L2)

            nc.sync.dma_start(out=out1[:], in_=outF1[:])
            nc.sync.dma_start(out=out2[:], in_=outF2[:])

    nc.compile()
    return nc


_NC_CACHE = {}


def _get_nc(mm_dtype=MM_DTYPE):
    if mm_dtype not in _NC_CACHE:
        _NC_CACHE[mm_dtype] = build_kernel(mm_dtype)
    return _NC_CACHE[mm_dtype]


def make_inputs(r1, r2, W1, b1, W2, b2, mm_dtype=MM_DTYPE):
    """Host-side shard + layout. Returns per-core input maps."""
    np_act = np.float32
    if mm_dtype == "bf16":
        import ml_dtypes
        np_act = ml_dtypes.bfloat16

    r1 = np.asarray(r1, dtype=np.float32)
    r2 = np.asarray(r2, dtype=np.float32)
    W1 = np.asarray(W1, dtype=np.float32)
    b1 = np.asarray(b1, dtype=np.float32)
    W2 = np.asarray(W2, dtype=np.float32)
    b2 = np.asarray(b2, dtype=np.float32)

    G = np.ascontiguousarray(W2.T @ W2, dtype=np.float32)
    u = np.ascontiguousarray(b2 @ W2, dtype=np.float32)

    # weight packs: lhsT[p, k, m] = M[k*128+p, m] with contraction d = k*128+p
    def pack(M):
        return np.ascontiguousarray(
            M.reshape(KT, 128, D).transpose(1, 0, 2), dtype=np_act)

    w1T = pack(W1.T)          # lhsT[d, m] = W1[m, d]
    gTp = pack(G)             # lhsT[d, m] = G[d, m]
    w2T = pack(W2.T)          # lhsT[d, m] = W2[m, d]
    b1d = np.ascontiguousarray(b1.reshape(KT, 128).T, dtype=np.float32)
    b2d = np.ascontiguousarray(b2.reshape(KT, 128).T, dtype=np.float32)
    uDd = np.ascontiguousarray(u.reshape(KT, 128).T, dtype=np_act)

    in_maps = []
    for c in range(NCORES):
        bs = slice(c * BL, (c + 1) * BL)
        a = r1[:, bs, :].transpose(2, 1, 0)          # (D, BL, L1)
        bt = r2[:, bs, :].transpose(2, 1, 0)         # (D, BL, L2)
        x = np.concatenate([a, bt], axis=2)          # (D, BL, NB)
        x = x.reshape(KT, 128, BL, NB).transpose(1, 0, 2, 3).reshape(
            128, KT, BL * NB)
        in_maps.append({
            "xT": np.ascontiguousarray(x, dtype=np_act),
            "w1T": w1T, "gT": gTp, "w2T": w2T,
            "b1d": b1d, "b2d": b2d, "uDd": uDd,
            "IDd": np.ascontiguousarray(np.eye(128, dtype=np.float32),
                                        dtype=np_act),
        })
    return in_maps


def _unpack(o):
    # device layout [128(p), BL, 2KT(f)] -> [BL, 2D] with d = f*128 + p
    return np.ascontiguousarray(o.transpose(1, 2, 0).reshape(BL, 2 * D))


def kernel(r1, r2, W1, b1, W2, b2):
    nc = _get_nc(MM_DTYPE)
    in_maps = make_inputs(r1, r2, W1, b1, W2, b2, MM_DTYPE)
    res = run_bass_kernel_spmd(nc, in_maps, core_ids=list(range(NCORES)))
    r1_pool = np.concatenate(
        [_unpack(res.results[c]["out1"]) for c in range(NCORES)], axis=0)
    r2_pool = np.concatenate(
        [_unpack(res.results[c]["out2"]) for c in range(NCORES)], axis=0)
    return (r1_pool, r2_pool)
